# revision 13
# baseline (speedup 1.0000x reference)
"""CostVolumeLoss Trainium2 kernel.

Computes: min over 5x5 window of per-pixel channel-mean L1 diff between
pred and zero-padded shifted target, then global mean. Data-parallel over
the batch dim (N=8) across 8 NeuronCores; each core reduces its image to a
single partial sum, host combines.

Per-core layout: H split into 4 tiles of 128 partitions, W in the free dim,
channels blocked in the free dim. For each dy in [-2,2] the target tile is
DMA-loaded row-shifted (zero halos memset), so every (dy,dx) offset becomes
a pure free-dim slice. Per offset: one tensor_tensor subtract, one fused
abs+channel-sum group tensor_reduce, one running-min tensor_tensor. The
last offset fuses the min with the spatial row-sum via
scalar_tensor_tensor(accum_out=...).
"""

import contextlib
import sys

if "/opt/trn_rl_repo" not in sys.path:
    sys.path.insert(0, "/opt/trn_rl_repo")

import numpy as np

import concourse.bass as bass
import concourse.mybir as mybir
from concourse.tile import TileContext
from concourse.bass_utils import run_bass_kernel_spmd

F32 = mybir.dt.float32
BF16 = mybir.dt.bfloat16
Op = mybir.AluOpType

N, C, H, W = 8, 3, 512, 512
KER = 5
PAD = (KER - 1) // 2  # 2
WP = W + 2 * PAD      # 516
NT = H // 128         # 4 H-tiles per core
NCORES = 8


def _split_waits(nc, maxw=1):
    """walrus in this env rejects >1 sem wait per instruction: split extra
    waits onto preceding NoOps on the same engine."""
    import bass_rust

    n = 0
    for fn in nc.m.functions:
        for blk in fn.blocks:
            out = []
            changed = False
            for inst in blk.instructions:
                si = inst.sync_info
                if si is not None and si.on_wait is not None and len(si.on_wait) > maxw:
                    waits = list(si.on_wait)
                    head, tail = waits[:-maxw], waits[-maxw:]
                    for i in range(0, len(head), maxw):
                        n += 1
                        out.append(
                            bass_rust.InstNoOp(
                                name=f"WSPLIT-{n}",
                                engine=inst.engine,
                                sync_info=mybir.SyncInfo(
                                    on_wait=head[i : i + maxw], on_update=[]
                                ),
                            )
                        )
                    si.on_wait = tail
                    changed = True
                out.append(inst)
            if changed:
                blk.instructions = out
    return n


def _emit_tile_pass_bf16(nc, pools, pred, targ, total, t, variant):
    """bf16 compute path: fp32 staging tiles are cast to bf16 on the Scalar
    engine, the 5x5 pass runs fully contiguous on DVE in bf16 (2x mode), the
    spatial row-sum upcasts back to fp32. variant 'bf16act' additionally
    moves the abs ops to the Scalar engine."""
    tpool, ppool, dpool, cpool, rpool = pools
    h0 = t * 128
    abs_on_act = variant == "bf16act"

    pstage = ppool.tile([128, C * W], F32, tag="pstage")
    nc.sync.dma_start(
        out=pstage[:].rearrange("p (c w) -> p c w", c=C),
        in_=pred[:, h0 : h0 + 128, :].rearrange("c h w -> h c w"),
    )
    ptile = ppool.tile([128, C * W], BF16, tag="ptile")
    nc.scalar.copy(ptile[:], pstage[:])
    pview = ptile[:].rearrange("p (c w) -> p c w", c=C)

    tviews = {}
    for dy in range(-PAD, PAD + 1):
        tstage = tpool.tile([128, C * WP], F32, tag="tstage")
        tsv = tstage[:].rearrange("p (c w) -> p c w", c=C)
        r0 = h0 + dy
        lo = max(0, r0)
        hi = min(H, r0 + 128)
        if lo > r0 or hi < r0 + 128:
            nc.vector.memset(tstage[:, :], 0.0)
        else:
            for c in range(C):
                nc.vector.memset(tsv[:, c, 0:PAD], 0.0)
                nc.vector.memset(tsv[:, c, PAD + W : WP], 0.0)
        nc.sync.dma_start(
            out=tsv[lo - r0 : hi - r0, :, PAD : PAD + W],
            in_=targ[:, lo:hi, :].rearrange("c h w -> h c w"),
        )
        ttb = tpool.tile([128, C * WP], BF16, tag="ttb")
        nc.scalar.copy(ttb[:], tstage[:])
        tviews[dy] = ttb[:].rearrange("p (c w) -> p c w", c=C)

    runmin = rpool.tile([128, W], BF16, tag="runminb")
    rowsum = rpool.tile([128, 1], F32, tag="rowsum")

    offsets = [
        (dy, dx) for dy in range(-PAD, PAD + 1) for dx in range(-PAD, PAD + 1)
    ]
    noff = len(offsets)
    for oi, (dy, dx) in enumerate(offsets):
        ttv = tviews[dy]
        d = dpool.tile([128, C * W], BF16, tag="db")
        tslice = ttv[:, :, PAD + dx : PAD + dx + W]
        csum = cpool.tile([128, W], BF16, tag="csumb")
        dv = d[:].rearrange("p (c w) -> p c w", c=C)
        for c in range(C):
            nc.vector.tensor_tensor(
                dv[:, c, :], pview[:, c, :], tslice[:, c, :], Op.subtract
            )
            if abs_on_act:
                nc.scalar.activation(
                    dv[:, c, :], dv[:, c, :], mybir.ActivationFunctionType.Abs
                )
            else:
                nc.vector.scalar_tensor_tensor(
                    dv[:, c, :], dv[:, c, :], -1.0, dv[:, c, :], Op.mult, Op.max
                )
        nc.vector.tensor_tensor(csum[:], dv[:, 0, :], dv[:, 1, :], Op.add)
        nc.vector.tensor_tensor(csum[:], csum[:], dv[:, 2, :], Op.add)
        if oi == 0:
            nc.vector.tensor_copy(runmin[:], csum[:])
        else:
            nc.vector.tensor_tensor(runmin[:], runmin[:], csum[:], Op.min)
    # row-sum with fp32 accumulation
    nc.vector.tensor_reduce(
        rowsum[:], runmin[:], mybir.AxisListType.X, Op.add
    )
    nc.vector.tensor_tensor(total[:], total[:], rowsum[:], Op.add)


def _emit_tile_pass(nc, pools, pred, targ, total, t, variant):
    """One H-tile (128 rows) of one image: full 5x5 cost-volume min pass,
    accumulating the spatial row-sums into `total`."""
    if variant in ("bf16", "bf16act"):
        return _emit_tile_pass_bf16(nc, pools, pred, targ, total, t, variant)
    tpool, ppool, dpool, cpool, rpool = pools
    h0 = t * 128

    ptile = ppool.tile([128, C * W], F32, tag="ptile")
    nc.sync.dma_start(
        out=ptile[:].rearrange("p (c w) -> p c w", c=C),
        in_=pred[:, h0 : h0 + 128, :].rearrange("c h w -> h c w"),
    )
    pview = ptile[:].rearrange("p (c w) -> p c w", c=C)

    tviews = {}
    dy_range = range(-PAD, PAD + 1) if variant != "noload" else [0]
    for dy in dy_range:
        tt = tpool.tile([128, C * WP], F32, tag="tt")
        ttv = tt[:].rearrange("p (c w) -> p c w", c=C)
        r0 = h0 + dy
        lo = max(0, r0)
        hi = min(H, r0 + 128)
        if lo > r0 or hi < r0 + 128:
            # engine partition starts must be 0/32/64/96: just zero the
            # whole tile on boundary tiles
            nc.vector.memset(tt[:, :], 0.0)
        else:
            for c in range(C):
                nc.vector.memset(ttv[:, c, 0:PAD], 0.0)
                nc.vector.memset(ttv[:, c, PAD + W : WP], 0.0)
        nc.sync.dma_start(
            out=ttv[lo - r0 : hi - r0, :, PAD : PAD + W],
            in_=targ[:, lo:hi, :].rearrange("c h w -> h c w"),
        )
        tviews[dy] = ttv
    if variant == "noload":
        for dy in range(-PAD, PAD + 1):
            tviews[dy] = tviews[0]

    runmin = rpool.tile([128, W], F32, tag="runmin")
    rowsum = rpool.tile([128, 1], F32, tag="rowsum")

    offsets = [
        (dy, dx) for dy in range(-PAD, PAD + 1) for dx in range(-PAD, PAD + 1)
    ]
    noff = len(offsets)
    for oi, (dy, dx) in enumerate(offsets):
        ttv = tviews[dy]
        d = dpool.tile([128, C * W], F32, tag="d")
        tslice = ttv[:, :, PAD + dx : PAD + dx + W]

        if variant in ("contig9", "contig9act"):
            # fully contiguous per-channel ops
            csum = cpool.tile([128, W], F32, tag="csum")
            dv = d[:].rearrange("p (c w) -> p c w", c=C)
            for c in range(C):
                nc.vector.tensor_tensor(
                    dv[:, c, :], pview[:, c, :], tslice[:, c, :], Op.subtract
                )
                if variant == "contig9act":
                    # abs on the otherwise-idle Scalar engine
                    nc.scalar.activation(
                        dv[:, c, :], dv[:, c, :],
                        mybir.ActivationFunctionType.Abs,
                    )
                else:
                    nc.vector.scalar_tensor_tensor(
                        dv[:, c, :], dv[:, c, :], -1.0, dv[:, c, :],
                        Op.mult, Op.max,
                    )
            nc.vector.tensor_tensor(csum[:], dv[:, 0, :], dv[:, 1, :], Op.add)
            nc.vector.tensor_tensor(csum[:], csum[:], dv[:, 2, :], Op.add)
            if oi == 0:
                nc.vector.tensor_copy(runmin[:], csum[:])
            elif oi < noff - 1:
                nc.vector.tensor_tensor(runmin[:], runmin[:], csum[:], Op.min)
            else:
                scratch = cpool.tile([128, W], F32, tag="scratch")
                nc.vector.scalar_tensor_tensor(
                    scratch[:], csum[:], 1.0, runmin[:],
                    Op.mult, Op.min, accum_out=rowsum[:],
                )
            continue

        if variant == "blocked":
            # d channel-blocked (contiguous write), reduce strided
            dout = d[:].rearrange("p (c w) -> p c w", c=C)
            dred = d[:].rearrange("p (c w) -> p w c", c=C)
        else:
            # d interleaved (c fastest): contiguous stream for the reduce
            dout = d[:].rearrange("p (w c) -> p c w", c=C)
            dred = d[:].rearrange("p (w c) -> p w c", c=C)
        nc.vector.tensor_tensor(dout, pview, tslice, Op.subtract)
        if oi == 0:
            nc.vector.tensor_reduce(
                runmin[:], dred, mybir.AxisListType.X, Op.add,
                apply_absolute_value=True,
            )
        else:
            csum = cpool.tile([128, W], F32, tag="csum")
            nc.vector.tensor_reduce(
                csum[:], dred, mybir.AxisListType.X, Op.add,
                apply_absolute_value=True,
            )
            if oi < noff - 1:
                nc.vector.tensor_tensor(runmin[:], runmin[:], csum[:], Op.min)
            else:
                # fused final min + spatial row-sum
                scratch = cpool.tile([128, W], F32, tag="scratch")
                nc.vector.scalar_tensor_tensor(
                    scratch[:], csum[:], 1.0, runmin[:],
                    Op.mult, Op.min, accum_out=rowsum[:],
                )
    nc.vector.tensor_tensor(total[:], total[:], rowsum[:], Op.add)


_LAST_EST_NS = None

T = 4                 # H blocks per image (partition dim = 128 rows each)
W2 = W + 2 * PAD      # 516 padded width
FDP = T * C * W       # 6144 pred free dim
FDT = T * C * W2      # 6192 padded target free dim
STG = FDT             # f32 stage width (4*3*516, full-tile rearrangeable)


def _v2_load_target(nc, spool, targ, dy, first_use, name=None, mset=None):
    """DMA one dy-shifted padded whole-image target into a rotating f32 stage.
    Layout [p, (t c w2)]: block t holds rows 128t+p+dy, zero-padded."""
    mset = mset or nc.vector
    st = spool.tile([128, STG], F32, name=name or f"st{dy}", tag="stage")
    stv = st[:].rearrange("p (t c w) -> p t c w", t=T, c=C)
    if first_use:
        mset.memset(st[:], 0.0)
    elif dy > 0:
        # reused buffer: rows >= H in block T-1 must be zero
        mset.memset(st[96:128, 3 * C * W2 : 4 * C * W2], 0.0)
    # DMA APs allow at most 3 dims: emit multi-block loads per channel
    # (out: p, t, w; in: p, t, w), boundary single-block loads with c folded.
    if dy < 0:
        nc.sync.dma_start(
            out=stv[-dy : 128, 0, :, PAD : PAD + W],
            in_=targ[:, 0 : 128 + dy, :].rearrange("c p w -> p c w"),
        )
        for c in range(C):
            nc.sync.dma_start(
                out=stv[:, 1:T, c, PAD : PAD + W],
                in_=targ[c, 128 + dy : 512 + dy, :].rearrange(
                    "(t p) w -> p t w", t=T - 1
                ),
            )
    elif dy == 0:
        for c in range(C):
            nc.sync.dma_start(
                out=stv[:, :, c, PAD : PAD + W],
                in_=targ[c].rearrange("(t p) w -> p t w", t=T),
            )
    else:
        for c in range(C):
            nc.sync.dma_start(
                out=stv[:, 0 : T - 1, c, PAD : PAD + W],
                in_=targ[c, dy : 384 + dy, :].rearrange(
                    "(t p) w -> p t w", t=T - 1
                ),
            )
        nc.sync.dma_start(
            out=stv[0 : 128 - dy, T - 1, :, PAD : PAD + W],
            in_=targ[:, 384 + dy : 512, :].rearrange("c p w -> p c w"),
        )
    return st


def _v2_cast(nc, eng, out_ap, in_ap):
    if eng == "dve":
        nc.vector.tensor_copy(out_ap, in_ap)
    elif eng == "gp":
        nc.gpsimd.tensor_copy(out_ap, in_ap)
    else:
        nc.scalar.copy(out_ap, in_ap)


def _build_v2(repeat=1, cfg=None):
    """Whole-image bf16 kernel: free dim folds (t=4 H-blocks, c, w).
    25 offsets; per offset: DVE sub (2x bf16), abs (ACT or DVE stt),
    DVE channel-sum adds + running min; final offset fuses min+row-sum."""
    global _LAST_EST_NS
    cfg = cfg or {}
    abs_eng = cfg.get("abs_eng", ["act"] * 25)
    cast_eng = cfg.get("cast_eng", ["dve"] * 11)
    sub_per_t = cfg.get("sub_per_t", False)
    min_eng = cfg.get("min_eng", ["dve"] * 25)
    add_eng = cfg.get("add_eng", ["dve"] * 25)
    skip_abs = cfg.get("skip_abs", False)
    skip_tail = cfg.get("skip_tail", False)
    skip_sub = cfg.get("skip_sub", False)

    nc = bass.Bass()
    pred = nc.declare_dram_parameter("pred", [C, H, W], F32, isOutput=False)
    targ = nc.declare_dram_parameter("target", [C, H, W], F32, isOutput=False)
    out = nc.declare_dram_parameter("out", [1, 1], F32, isOutput=True)

    with TileContext(nc) as tc:
        with (
            tc.tile_pool(name="spool", bufs=2) as spool,
            tc.tile_pool(name="bpool", bufs=1) as bpool,
            tc.tile_pool(name="opool", bufs=1) as opool,
            tc.tile_pool(name="dpool", bufs=3) as dpool,
            tc.tile_pool(name="cpool", bufs=3) as cpool,
            tc.tile_pool(name="rpool", bufs=1) as rpool,
        ):
            runmin = rpool.tile([128, T * W], BF16, name="runmin")
            rowsum = rpool.tile([128, 1], F32, name="rowsum")
            red = rpool.tile([1, 1], F32, name="red")
            if skip_tail:
                nc.vector.memset(runmin[:], 0.0)
                nc.vector.memset(rowsum[:], 0.0)

            loop_ctx = (
                tc.For_i(0, repeat, 1) if repeat > 1 else contextlib.nullcontext()
            )
            with loop_ctx:
                ci = iter(cast_eng)
                # pred load + cast (same padded layout as target; halos unread)
                pst = _v2_load_target(nc, spool, pred, 0, False, name="pst")
                pbf = bpool.tile([128, FDT], BF16, name="pbf")
                _v2_cast(nc, next(ci), pbf[:], pst[:])
                pview = pbf[:].rearrange("p (t c w) -> p t c w", t=T, c=C)[
                    :, :, :, PAD : PAD + W
                ]

                tviews = {}
                def load_dy(dy, first_use):
                    st = _v2_load_target(nc, spool, targ, dy, first_use)
                    tb = bpool.tile([128, FDT], BF16, name=f"tb{dy}")
                    _v2_cast(nc, next(ci), tb[:], st[:])
                    tbo = opool.tile([128, FDT], BF16, name=f"tbo{dy}", tag="tbo")
                    _v2_cast(nc, next(ci), tbo[:, 0 : FDT - 2], st[:, 1 : FDT - 1])
                    tviews[dy] = (
                        tb[:].rearrange("p (t c w) -> p t c w", t=T, c=C),
                        tbo[:].rearrange("p (t c w) -> p t c w", t=T, c=C),
                    )

                load_dy(-2, True)
                load_dy(-1, True)

                offsets = [
                    (dy, dx)
                    for dy in range(-PAD, PAD + 1)
                    for dx in range(-PAD, PAD + 1)
                ]

                def emit_tail(oi, d):
                    """channel-sum + running min for offset oi (d = |diff|)."""
                    dv = d[:].rearrange("p (t c w) -> p t c w", t=T, c=C)
                    d0, d1, d2 = dv[:, :, 0, :], dv[:, :, 1, :], dv[:, :, 2, :]
                    veng = nc.vector if add_eng[oi] == "dve" else nc.gpsimd
                    meng = nc.vector if min_eng[oi] == "dve" else nc.gpsimd
                    if oi == 0:
                        rv = runmin[:].rearrange("p (t w) -> p t w", t=T)
                        veng.tensor_tensor(rv, d0, d1, Op.add)
                        nc.vector.tensor_tensor(rv, rv, d2, Op.add)
                        return
                    cs = cpool.tile([128, T * W], BF16, name=f"cs{oi}", tag="cs")
                    cv = cs[:].rearrange("p (t w) -> p t w", t=T)
                    veng.tensor_tensor(cv, d0, d1, Op.add)
                    nc.vector.tensor_tensor(cv, cv, d2, Op.add)
                    if oi < len(offsets) - 1:
                        meng.tensor_tensor(runmin[:], runmin[:], cs[:], Op.min)
                    else:
                        scratch = cpool.tile(
                            [128, T * W], BF16, name="scratch", tag="cs"
                        )
                        nc.vector.scalar_tensor_tensor(
                            scratch[:], cs[:], 1.0, runmin[:],
                            Op.mult, Op.min, accum_out=rowsum[:],
                        )

                pending = None
                for oi, (dy, dx) in enumerate(offsets):
                    if dx == -PAD and dy + 1 <= PAD and oi > 0:
                        load_dy(dy + 1, dy + 1 <= 0)
                    tbv, tbov = tviews[dy]
                    if dx % 2 == 0:
                        tsl = tbv[:, :, :, PAD + dx : PAD + dx + W]
                    else:
                        tsl = tbov[:, :, :, PAD + dx - 1 : PAD + dx - 1 + W]
                    d = dpool.tile([128, FDP], BF16, name=f"d{oi}", tag="d")
                    dv = d[:].rearrange("p (t c w) -> p t c w", t=T, c=C)
                    if skip_sub:
                        if oi < 3:
                            nc.vector.memset(d[:], 0.0)
                    elif sub_per_t:
                        for t in range(T):
                            nc.vector.tensor_tensor(
                                dv[:, t], pview[:, t], tsl[:, t], Op.subtract
                            )

                    else:
                        nc.vector.tensor_tensor(dv, pview, tsl, Op.subtract)
                    if skip_abs:
                        pass
                    elif abs_eng[oi] == "act":
                        nc.scalar.activation(
                            d[:], d[:], mybir.ActivationFunctionType.Abs
                        )
                    else:
                        nc.vector.scalar_tensor_tensor(
                            d[:], d[:], -1.0, d[:], Op.mult, Op.max
                        )
                    if pending is not None and not skip_tail:
                        emit_tail(*pending)
                    pending = (oi, d)
                if not skip_tail:
                    emit_tail(*pending)

            nc.gpsimd.tensor_reduce(
                red[:], rowsum[:], mybir.AxisListType.C, Op.add
            )
            nc.sync.dma_start(out=out[0:1, 0:1], in_=red[:])

    _LAST_EST_NS = (
        max(e[2] for e in tc._perfetto_entries) if tc._perfetto_entries else None
    )
    _split_waits(nc, 1)
    return nc




def _build_v3(repeat=1, cfg=None):
    """v2 + offset pairing: two offsets share one double-wide d tile so the
    channel-sum adds and running-min ops cover 2 offsets per instruction
    (FD 4096), halving DVE per-op overhead. Memsets+casts go to GPSIMD.
    tb tiles rotate (3 bufs); d tiles rotate (skew-2 pipeline)."""
    global _LAST_EST_NS
    cfg = cfg or {}
    npair = 13
    abs_eng = cfg.get("abs_eng", ["act"] * npair)
    cast_eng = cfg.get("cast_eng", ["gp"] * 11)
    mset_eng = cfg.get("mset_eng", "gp")
    skew = cfg.get("skew", 2)
    dbufs = cfg.get("dbufs", 3)
    flat_pred = cfg.get("flat_pred", True)
    no_tbo = cfg.get("no_tbo", False)

    nc = bass.Bass()
    pred = nc.declare_dram_parameter("pred", [C, H, W], F32, isOutput=False)
    targ = nc.declare_dram_parameter("target", [C, H, W], F32, isOutput=False)
    out = nc.declare_dram_parameter("out", [1, 1], F32, isOutput=True)

    with TileContext(nc) as tc:
        with (
            tc.tile_pool(name="spool", bufs=2) as spool,
            tc.tile_pool(name="ppool", bufs=1) as ppool,
            tc.tile_pool(name="tbpool", bufs=3) as tbpool,
            tc.tile_pool(name="opool", bufs=1) as opool,
            tc.tile_pool(name="dpool", bufs=dbufs) as dpool,
            tc.tile_pool(name="cpool", bufs=2) as cpool,
            tc.tile_pool(name="rpool", bufs=1) as rpool,
        ):
            runmin2 = rpool.tile([128, 2 * T * W], BF16, name="runmin2")
            rowsum = rpool.tile([128, 1], F32, name="rowsum")
            red = rpool.tile([1, 1], F32, name="red")
            meng = nc.gpsimd if mset_eng == "gp" else nc.vector

            loop_ctx = (
                tc.For_i(0, repeat, 1) if repeat > 1 else contextlib.nullcontext()
            )
            with loop_ctx:
                ci = iter(cast_eng)
                if flat_pred:
                    # unpadded pred: flat 6144 free dim, 2-dim sub in0 AP
                    pst = spool.tile([128, STG], F32, name="pst", tag="stage")
                    psv = pst[:, 0 : T * C * W].rearrange(
                        "p (t c w) -> p t c w", t=T, c=C
                    )
                    for c in range(C):
                        nc.sync.dma_start(
                            out=psv[:, :, c, :],
                            in_=pred[c].rearrange("(t p) w -> p t w", t=T),
                        )
                    pbf = ppool.tile([128, T * C * W], BF16, name="pbf")
                    _v2_cast(nc, next(ci), pbf[:], pst[:, 0 : T * C * W])
                    pview = pbf[:].rearrange("p (t c w) -> p t c w", t=T, c=C)
                else:
                    pst = _v2_load_target(nc, spool, pred, 0, False, name="pst",
                                          mset=meng)
                    pbf = ppool.tile([128, FDT], BF16, name="pbf")
                    _v2_cast(nc, next(ci), pbf[:], pst[:])
                    pview = pbf[:].rearrange("p (t c w) -> p t c w", t=T, c=C)[
                        :, :, :, PAD : PAD + W
                    ]

                tviews = {}
                def load_dy(dy, first_use):
                    st = _v2_load_target(nc, spool, targ, dy, first_use,
                                         mset=meng)
                    tb = tbpool.tile([128, FDT], BF16, name=f"tb{dy}", tag="tb")
                    _v2_cast(nc, next(ci), tb[:], st[:])
                    tbv = tb[:].rearrange("p (t c w) -> p t c w", t=T, c=C)
                    if no_tbo:
                        tviews[dy] = (tbv, tbv)
                    else:
                        tbo = opool.tile(
                            [128, FDT], BF16, name=f"tbo{dy}", tag="tbo"
                        )
                        _v2_cast(
                            nc, next(ci), tbo[:, 0 : FDT - 2], st[:, 1 : FDT - 1]
                        )
                        tviews[dy] = (
                            tbv,
                            tbo[:].rearrange("p (t c w) -> p t c w", t=T, c=C),
                        )

                load_dy(-2, True)
                load_dy(-1, True)

                offsets = [
                    (dy, dx)
                    for dy in range(-PAD, PAD + 1)
                    for dx in range(-PAD, PAD + 1)
                ]

                def emit_sub(oi, dm, half):
                    dy, dx = offsets[oi]
                    if dx == PAD and 0 <= dy + 1 <= PAD:
                        load_dy(dy + 1, False)
                    tbv, tbov = tviews[dy]
                    if no_tbo or dx % 2 == 0:
                        tsl = tbv[:, :, :, PAD + dx : PAD + dx + W]
                    else:
                        tsl = tbov[:, :, :, PAD + dx - 1 : PAD + dx - 1 + W]
                    dv = dm[:, half * FDP : (half + 1) * FDP].rearrange(
                        "p (t c w) -> p t c w", t=T, c=C
                    )
                    nc.vector.tensor_tensor(dv, pview, tsl, Op.subtract)

                def emit_abs(pi, dm, n):
                    if abs_eng[pi] == "act":
                        nc.scalar.activation(
                            dm[:, 0 : n * FDP], dm[:, 0 : n * FDP],
                            mybir.ActivationFunctionType.Abs,
                        )
                    else:
                        nc.vector.scalar_tensor_tensor(
                            dm[:, 0 : n * FDP], dm[:, 0 : n * FDP], -1.0,
                            dm[:, 0 : n * FDP], Op.mult, Op.max,
                        )

                def emit_tail(pi, dm, n):
                    """channel-sum + running min for pair pi over n offsets."""
                    dv = dm[:].rearrange(
                        "p (o t c w) -> p o t c w", o=2, t=T, c=C
                    )
                    d0, d1, d2 = (dv[:, 0:n, :, c, :] for c in range(C))
                    if pi == 0:
                        rv = runmin2[:].rearrange(
                            "p (o t w) -> p o t w", o=2, t=T
                        )
                        nc.vector.tensor_tensor(rv, d0, d1, Op.add)
                        nc.vector.tensor_tensor(rv, rv, d2, Op.add)
                        return
                    cs = cpool.tile([128, 2 * T * W], BF16, name=f"cs{pi}", tag="cs")
                    cv = cs[:].rearrange("p (o t w) -> p o t w", o=2, t=T)[:, 0:n]
                    nc.vector.tensor_tensor(cv, d0, d1, Op.add)
                    nc.vector.tensor_tensor(cv, cv, d2, Op.add)
                    if pi < npair - 1:
                        nc.vector.tensor_tensor(
                            runmin2[:, 0 : n * T * W], runmin2[:, 0 : n * T * W],
                            cs[:, 0 : n * T * W], Op.min,
                        )
                    else:
                        nc.vector.tensor_tensor(
                            runmin2[:, 0 : T * W], runmin2[:, 0 : T * W],
                            runmin2[:, T * W : 2 * T * W], Op.min,
                        )
                        scratch = cpool.tile(
                            [128, 2 * T * W], BF16, name="scratch", tag="cs"
                        )
                        nc.vector.scalar_tensor_tensor(
                            scratch[:, 0 : T * W], cs[:, 0 : T * W], 1.0,
                            runmin2[:, 0 : T * W], Op.mult, Op.min,
                            accum_out=rowsum[:],
                        )

                from collections import deque
                pending = deque()
                for pi in range(npair):
                    n = 2 if 2 * pi + 1 < len(offsets) else 1
                    dm = dpool.tile([128, 2 * FDP], BF16, name=f"dm{pi}", tag="d")
                    emit_sub(2 * pi, dm, 0)
                    if n == 2:
                        emit_sub(2 * pi + 1, dm, 1)
                    emit_abs(pi, dm, n)
                    pending.append((pi, dm, n))
                    if len(pending) > skew:
                        emit_tail(*pending.popleft())
                while pending:
                    emit_tail(*pending.popleft())

            nc.gpsimd.tensor_reduce(
                red[:], rowsum[:], mybir.AxisListType.C, Op.add
            )
            nc.sync.dma_start(out=out[0:1, 0:1], in_=red[:])

    _LAST_EST_NS = (
        max(e[2] for e in tc._perfetto_entries) if tc._perfetto_entries else None
    )
    _split_waits(nc, 1)
    return nc


def _build(repeat=1, variant="base", cfg=None):
    if variant == "v2":
        return _build_v2(repeat=repeat, cfg=cfg)
    if variant == "v3":
        return _build_v3(repeat=repeat, cfg=cfg)
    global _LAST_EST_NS
    nc = bass.Bass()
    pred = nc.declare_dram_parameter("pred", [C, H, W], F32, isOutput=False)
    targ = nc.declare_dram_parameter("target", [C, H, W], F32, isOutput=False)
    out = nc.declare_dram_parameter("out", [1, 1], F32, isOutput=True)

    with TileContext(nc) as tc:
        with (
            tc.tile_pool(name="tpool", bufs=10) as tpool,
            tc.tile_pool(name="ppool", bufs=2) as ppool,
            tc.tile_pool(name="dpool", bufs=3) as dpool,
            tc.tile_pool(name="cpool", bufs=3) as cpool,
            tc.tile_pool(name="rpool", bufs=2) as rpool,
            tc.tile_pool(name="spool", bufs=1) as spool,
        ):
            total = spool.tile([128, 1], F32)
            nc.vector.memset(total[:], 0.0)

            pools = (tpool, ppool, dpool, cpool, rpool)
            loop_ctx = (
                tc.For_i(0, repeat, 1) if repeat > 1 else contextlib.nullcontext()
            )
            with loop_ctx:
                for t in range(NT):
                    _emit_tile_pass(nc, pools, pred, targ, total, t, variant)

            red = spool.tile([1, 1], F32)
            nc.gpsimd.tensor_reduce(red[:], total[:], mybir.AxisListType.C, Op.add)
            nc.sync.dma_start(out=out[0:1, 0:1], in_=red[:])

    _LAST_EST_NS = (
        max(e[2] for e in tc._perfetto_entries) if tc._perfetto_entries else None
    )

    _split_waits(nc, 1)
    return nc


_NC_CACHE = None


KERNEL_VARIANT = "v3"
KERNEL_CFG = None


def _get_nc():
    global _NC_CACHE
    if _NC_CACHE is None:
        _NC_CACHE = _build(variant=KERNEL_VARIANT, cfg=KERNEL_CFG)
    return _NC_CACHE


def kernel(pred, target_warpped, _trace=False):
    pred = np.ascontiguousarray(np.asarray(pred, dtype=np.float32))
    targ = np.ascontiguousarray(np.asarray(target_warpped, dtype=np.float32))
    assert pred.shape == (N, C, H, W) and targ.shape == (N, C, H, W)

    nc = _get_nc()
    in_maps = [
        {"pred": np.ascontiguousarray(pred[i]), "target": np.ascontiguousarray(targ[i])}
        for i in range(NCORES)
    ]
    res = run_bass_kernel_spmd(nc, in_maps, core_ids=list(range(NCORES)), trace=_trace)
    partials = np.array(
        [res.results[i]["out"][0, 0] for i in range(NCORES)], dtype=np.float64
    )
    loss = partials.sum() / (C * N * H * W)
    out = np.float32(loss)
    if _trace:
        return out, res
    return out

